# revision 16
# baseline (speedup 1.0000x reference)
"""MoE (8 experts, top-2) Trainium2 kernel.

Strategy (expert parallelism, per the sharding hint):
  - Host: compute the (tiny) gate: logits -> softmax -> top-2 -> normalized
    combine weights.  This is ~0.05% of total FLOPs.
  - Dispatch: for each expert e, gather the tokens routed to it (~2k of
    8192 * 2 slots), pad to a common capacity C, and send them (transposed,
    [EMB, C]) to core e together with expert e's weights.
  - Device (SPMD on 8 cores): dense FFN  y^T = W2^T @ gelu(W1^T @ x^T + b1) + b2
    computed block-by-block over tokens with all matmuls on the PE array.
  - Combine: host scales each expert's output rows by the combine weight and
    scatter-adds into the full output (each token receives exactly its top-2
    expert contributions).  aux_loss is reproduced on host from gate scores.

Only the FFN matmuls (99.9% of FLOPs) run on device; they are sized
[C,1024]x[1024,4096] + [C,4096]x[4096,1024] per core.
"""

import os
import numpy as np

import concourse.bass as bass
import concourse.mybir as mybir
from concourse import bacc
from concourse.bass import ds
from concourse.tile import TileContext
from concourse.bass_utils import run_bass_kernel_spmd

EMB = 1024
FF = 4096
NE = 8
TOPK = 2
P = 128

TB = 768        # tokens per block
MM_N = 384      # matmul moving free dim (TB == 2 * MM_N)
NH = TB // MM_N
KD = EMB // P   # 8 contraction tiles over EMB
KF = FF // P    # 32 contraction tiles over FF
FG = 2          # ff-tiles of W1 per streamed chunk (phase A)
FG2 = 4         # ff-tiles of W2 per streamed chunk (phase B)
MQ = 2          # emb-tiles per phase-B pass (4 passes x 2 m-tiles)

# matmul precision: "f32" (exact, 4 cyc/row), "f32r" (fp32 storage,
# relaxed multiply, 1 cyc/row), "bf16" (bf16 storage+multiply, 1 cyc/row)
MM_DTYPE = os.environ.get("MOE_MM_DTYPE", "f32r")

_ACT_FUNC = mybir.ActivationFunctionType.Gelu  # swapped to Relu in sim tests

PROFILE = False          # set True (from test.py) to capture an NTFF trace
PROFILE_ALL_CORES = False
LAST_RESULTS = None      # BassKernelResults of the last run when PROFILE

_NC_CACHE = {}


def _build(C: int, tag: str) -> bass.Bass:
    f32 = mybir.dt.float32
    io_dt = {
        "bf16": mybir.dt.bfloat16,
        "f32r": mybir.dt.float32r,  # fp32 container, relaxed-precision multiply
        "f32": f32,
    }[tag]
    mmcast = lambda ap: ap

    NB = C // TB
    assert C % TB == 0
    bf16 = tag == "bf16"
    FG_ = 4 if bf16 else FG          # W1 stream chunk (ff-tiles)
    w1_bufs = 3
    w2_bufs = 8 if bf16 else 6
    h_bufs = 2 if bf16 else 1

    nc = bacc.Bacc()
    xt_d = nc.declare_dram_parameter("xt", [EMB, C], io_dt, isOutput=False)
    w1_d = nc.declare_dram_parameter("w1", [EMB, FF], io_dt, isOutput=False)
    b1_d = nc.declare_dram_parameter("b1", [FF], f32, isOutput=False)
    w2_d = nc.declare_dram_parameter("w2", [FF, EMB], io_dt, isOutput=False)
    b2_d = nc.declare_dram_parameter("b2", [EMB], f32, isOutput=False)
    yt_d = nc.declare_dram_parameter("yt", [EMB, C], f32, isOutput=True)

    xt_r = xt_d[:].rearrange("(ko p) c -> p ko c", p=P)   # [128, 8, C]
    w1_r = w1_d[:].rearrange("(ko p) f -> p ko f", p=P)   # [128, 8, FF]
    w2_r = w2_d[:].rearrange("(ko p) e -> p ko e", p=P)   # [128, 32, EMB]
    b1_r = b1_d[:].rearrange("(t p) -> p t", p=P)         # [128, 32]
    b2_r = b2_d[:].rearrange("(t p) -> p t", p=P)         # [128, 8]
    yt_r = yt_d[:].rearrange("(mo p) c -> p mo c", p=P)   # [128, 8, C]

    with TileContext(nc) as tc:
        with (
            tc.tile_pool(name="const", bufs=1) as cpool,
            tc.tile_pool(name="xt", bufs=1) as xt_pool,
            tc.tile_pool(name="w1", bufs=w1_bufs) as w1_pool,
            tc.tile_pool(name="w2", bufs=w2_bufs) as w2_pool,
            tc.tile_pool(name="h", bufs=h_bufs) as h_pool,
            tc.tile_pool(name="y", bufs=4) as y_pool,
            tc.tile_pool(name="psA", bufs=2, space="PSUM") as psA,
            tc.tile_pool(name="psB", bufs=6, space="PSUM") as psB,
        ):
            b1_sb = cpool.tile([P, KF], f32)
            nc.sync.dma_start(b1_sb, b1_r)
            b2_sb = cpool.tile([P, KD], f32)
            nc.sync.dma_start(b2_sb, b2_r)

            def load_xt(blk):
                # per-k DMAs spread across queues so the first matmuls of a
                # block don't wait for one monolithic transfer
                t = xt_pool.tile([P, KD, TB], io_dt, name=f"xt{blk}", tag="xt")
                for k in range(KD):
                    nc.sync.dma_start(t[:, k], xt_r[:, k, ds(blk * TB, TB)])
                return t

            def load_w2_first(blk):
                # preload the first W2 chunk of every mq pass during phase A
                pre = []
                for mq in range(KD // MQ):
                    t = w2_pool.tile(
                        [P, FG2, MQ * P], io_dt, name=f"w2p{mq}", tag="w2"
                    )
                    nc.sync.dma_start(
                        t, w2_r[:, ds(0, FG2), ds(mq * MQ * P, MQ * P)]
                    )
                    pre.append(t)
                return pre

            xt_sb = load_xt(0)
            for blk in range(NB):
                h_sb = h_pool.tile([P, KF, TB], io_dt)
                w2_pre = None

                # ---- phase A: h^T[f, :] = gelu(W1^T x^T + b1) --------------
                for fg in range(KF // FG_):
                    w1_sb = w1_pool.tile([P, KD, FG_ * P], io_dt)
                    for k in range(KD):
                        nc.sync.dma_start(
                            w1_sb[:, k], w1_r[:, k, ds(fg * FG_ * P, FG_ * P)]
                        )
                    for fi in range(FG_):
                        f = fg * FG_ + fi
                        for nh in range(NH):
                            pa = psA.tile([P, MM_N], f32)
                            for k in range(KD):
                                nc.tensor.matmul(
                                    pa,
                                    mmcast(w1_sb[:, k, ds(fi * P, P)]),
                                    mmcast(xt_sb[:, k, ds(nh * MM_N, MM_N)]),
                                    start=(k == 0),
                                    stop=(k == KD - 1),
                                )
                            nc.scalar.activation(
                                h_sb[:, f, ds(nh * MM_N, MM_N)],
                                pa,
                                _ACT_FUNC,
                                bias=b1_sb[:, f : f + 1],
                            )
                    if fg == 1:
                        w2_pre = load_w2_first(blk)

                # next block's tokens: DMA starts as soon as phase A's last
                # read of the current xt slot retires, landing early in B
                xt_next = load_xt(blk + 1) if blk + 1 < NB else None

                # ---- phase B: y^T[m, :] = W2^T h^T + b2 --------------------
                for mq in range(KD // MQ):
                    pb = [
                        psB.tile([P, MM_N], f32, name=f"pb{i}", tag="pb")
                        for i in range(MQ * NH)
                    ]
                    for fg2 in range(KF // FG2):
                        if fg2 == 0:
                            w2_sb = w2_pre[mq]
                        else:
                            w2_sb = w2_pool.tile([P, FG2, MQ * P], io_dt)
                            nc.sync.dma_start(
                                w2_sb,
                                w2_r[:, ds(fg2 * FG2, FG2), ds(mq * MQ * P, MQ * P)],
                            )
                        for fi in range(FG2):
                            ffk = fg2 * FG2 + fi
                            for mm in range(MQ):
                                for nh in range(NH):
                                    nc.tensor.matmul(
                                        pb[mm * NH + nh],
                                        mmcast(w2_sb[:, fi, ds(mm * P, P)]),
                                        mmcast(h_sb[:, ffk, ds(nh * MM_N, MM_N)]),
                                        start=(ffk == 0),
                                        stop=(ffk == KF - 1),
                                    )
                    for mm in range(MQ):
                        m = mq * MQ + mm
                        for nh in range(NH):
                            y_sb = y_pool.tile([P, MM_N], f32)
                            nc.vector.tensor_scalar_add(
                                y_sb, pb[mm * NH + nh], b2_sb[:, m : m + 1]
                            )
                            nc.sync.dma_start(
                                yt_r[:, m, ds(blk * TB + nh * MM_N, MM_N)], y_sb
                            )
                xt_sb = xt_next
    if not nc.is_finalized():
        nc.finalize()
    return nc


def _get_nc(C: int, tag: str) -> bass.Bass:
    key = (C, tag)
    if key not in _NC_CACHE:
        _NC_CACHE[key] = _build(C, tag)
    return _NC_CACHE[key]


def kernel(x, gate_W, gate_b, W1, b1, W2, b2):
    global LAST_RESULTS
    x = np.asarray(x, np.float32)
    gate_W = np.asarray(gate_W, np.float32)
    gate_b = np.asarray(gate_b, np.float32)
    W1 = np.asarray(W1, np.float32)
    b1 = np.asarray(b1, np.float32)
    W2 = np.asarray(W2, np.float32)
    b2 = np.asarray(b2, np.float32)

    B, S, D = x.shape
    T = B * S
    xf = x.reshape(T, D)

    # ---- gate / routing (host) ------------------------------------------
    logits = xf @ gate_W + gate_b
    m = logits.max(-1, keepdims=True)
    ex = np.exp(logits - m)
    scores = ex / ex.sum(-1, keepdims=True)          # [T, NE] float32
    ar = np.arange(T)
    idx0 = scores.argmax(-1)
    tmp = scores.copy()
    tmp[ar, idx0] = -np.inf
    idx1 = tmp.argmax(-1)
    s0 = scores[ar, idx0]
    s1 = scores[ar, idx1]
    den = s0 + s1
    w0 = s0 / den
    w1w = s1 / den

    tok_idx = [None] * NE
    tok_w = [None] * NE
    for e in range(NE):
        m0 = idx0 == e
        m1 = idx1 == e
        ids = np.concatenate([ar[m0], ar[m1]])
        ws = np.concatenate([w0[m0], w1w[m1]]).astype(np.float32)
        tok_idx[e] = ids
        tok_w[e] = ws

    n_max = max(len(t) for t in tok_idx)
    C = TB * ((n_max + TB - 1) // TB)

    tag = MM_DTYPE
    io_np = np.float32
    if tag == "bf16":
        import ml_dtypes
        io_np = ml_dtypes.bfloat16

    # ---- build per-core inputs ------------------------------------------
    in_maps = []
    for e in range(NE):
        ids = tok_idx[e]
        XT = np.zeros((EMB, C), io_np)
        XT[:, : len(ids)] = xf[ids].T
        in_maps.append(
            {
                "xt": XT,
                "w1": np.ascontiguousarray(W1[e]).astype(io_np),
                "b1": np.ascontiguousarray(b1[e]),
                "w2": np.ascontiguousarray(W2[e]).astype(io_np),
                "b2": np.ascontiguousarray(b2[e]),
            }
        )

    nc = _get_nc(C, tag)
    res = run_bass_kernel_spmd(
        nc,
        in_maps,
        list(range(NE)),
        trace=PROFILE,
        trace_cores=list(range(NE)) if (PROFILE and PROFILE_ALL_CORES) else None,
    )
    if PROFILE:
        LAST_RESULTS = res

    # ---- combine (host) --------------------------------------------------
    out = np.zeros((T, EMB), np.float32)
    for e in range(NE):
        ids = tok_idx[e]
        n = len(ids)
        Y = res.results[e]["yt"][:, :n].T          # [n, EMB]
        out[ids] += tok_w[e][:, None] * Y

    # ---- aux loss (host) -------------------------------------------------
    counts = np.zeros(NE, np.float64)
    np.add.at(counts, idx0, 1.0)
    np.add.at(counts, idx1, 1.0)
    expert_fraction = counts / (T * TOPK)
    routing_weights = scores.mean(0, dtype=np.float64)
    aux_loss = np.float32(NE * np.sum(expert_fraction * routing_weights))

    return out.reshape(B, S, EMB), aux_loss


# revision 17
# speedup vs baseline: 1.0734x; 1.0734x over previous
"""MoE (8 experts, top-2) Trainium2 kernel.

Strategy (expert parallelism, per the sharding hint):
  - Host: compute the (tiny) gate: logits -> softmax -> top-2 -> normalized
    combine weights.  This is ~0.05% of total FLOPs.
  - Dispatch: for each expert e, gather the tokens routed to it (~2k of
    8192 * 2 slots), pad to a common capacity C, and send them (transposed,
    [EMB, C]) to core e together with expert e's weights.
  - Device (SPMD on 8 cores): dense FFN  y^T = W2^T @ gelu(W1^T @ x^T + b1) + b2
    computed block-by-block over tokens with all matmuls on the PE array.
  - Combine: host scales each expert's output rows by the combine weight and
    scatter-adds into the full output (each token receives exactly its top-2
    expert contributions).  aux_loss is reproduced on host from gate scores.

Only the FFN matmuls (99.9% of FLOPs) run on device; they are sized
[C,1024]x[1024,4096] + [C,4096]x[4096,1024] per core.
"""

import os
import numpy as np

import concourse.bass as bass
import concourse.mybir as mybir
from concourse import bacc
from concourse.bass import ds
from concourse.tile import TileContext
from concourse.bass_utils import run_bass_kernel_spmd

EMB = 1024
FF = 4096
NE = 8
TOPK = 2
P = 128

TB = 768        # tokens per block
MM_N = 384      # matmul moving free dim (TB == 2 * MM_N)
NH = TB // MM_N
KD = EMB // P   # 8 contraction tiles over EMB
KF = FF // P    # 32 contraction tiles over FF
FG = 2          # ff-tiles of W1 per streamed chunk (phase A)
FG2 = 4         # ff-tiles of W2 per streamed chunk (phase B)
MQ = 2          # emb-tiles per phase-B pass (4 passes x 2 m-tiles)

# matmul precision: "f32" (exact, 4 cyc/row), "f32r" (fp32 storage,
# relaxed multiply, 1 cyc/row), "bf16" (bf16 storage+multiply, 1 cyc/row)
MM_DTYPE = os.environ.get("MOE_MM_DTYPE", "f32r")

_ACT_FUNC = mybir.ActivationFunctionType.Gelu  # swapped to Relu in sim tests

PROFILE = False          # set True (from test.py) to capture an NTFF trace
PROFILE_ALL_CORES = False
LAST_RESULTS = None      # BassKernelResults of the last run when PROFILE

_NC_CACHE = {}


def _build(C: int, tag: str) -> bass.Bass:
    f32 = mybir.dt.float32
    io_dt = {
        "bf16": mybir.dt.bfloat16,
        "f32r": mybir.dt.float32r,  # fp32 container, relaxed-precision multiply
        "f32": f32,
    }[tag]
    mmcast = lambda ap: ap

    NB = C // TB
    assert C % TB == 0
    bf16 = tag == "bf16"
    FG_ = 4                          # W1 stream chunk (ff-tiles)
    w1_bufs = 3 if bf16 else 2
    w2_bufs = 8 if bf16 else 6
    h_bufs = 2 if bf16 else 1

    nc = bacc.Bacc()
    xt_d = nc.declare_dram_parameter("xt", [EMB, C], io_dt, isOutput=False)
    w1_d = nc.declare_dram_parameter("w1", [EMB, FF], io_dt, isOutput=False)
    b1_d = nc.declare_dram_parameter("b1", [FF], f32, isOutput=False)
    w2_d = nc.declare_dram_parameter("w2", [FF, EMB], io_dt, isOutput=False)
    b2_d = nc.declare_dram_parameter("b2", [EMB], f32, isOutput=False)
    yt_d = nc.declare_dram_parameter("yt", [EMB, C], f32, isOutput=True)

    xt_r = xt_d[:].rearrange("(ko p) c -> p ko c", p=P)   # [128, 8, C]
    w1_r = w1_d[:].rearrange("(ko p) f -> p ko f", p=P)   # [128, 8, FF]
    w2_r = w2_d[:].rearrange("(ko p) e -> p ko e", p=P)   # [128, 32, EMB]
    b1_r = b1_d[:].rearrange("(t p) -> p t", p=P)         # [128, 32]
    b2_r = b2_d[:].rearrange("(t p) -> p t", p=P)         # [128, 8]
    yt_r = yt_d[:].rearrange("(mo p) c -> p mo c", p=P)   # [128, 8, C]

    with TileContext(nc) as tc:
        with (
            tc.tile_pool(name="const", bufs=1) as cpool,
            tc.tile_pool(name="xt", bufs=1) as xt_pool,
            tc.tile_pool(name="w1", bufs=w1_bufs) as w1_pool,
            tc.tile_pool(name="w2", bufs=w2_bufs) as w2_pool,
            tc.tile_pool(name="h", bufs=h_bufs) as h_pool,
            tc.tile_pool(name="y", bufs=4) as y_pool,
            tc.tile_pool(name="psA", bufs=2, space="PSUM") as psA,
            tc.tile_pool(name="psB", bufs=6, space="PSUM") as psB,
        ):
            b1_sb = cpool.tile([P, KF], f32)
            nc.sync.dma_start(b1_sb, b1_r)
            b2_sb = cpool.tile([P, KD], f32)
            nc.sync.dma_start(b2_sb, b2_r)

            def load_xt(blk):
                # per-k DMAs spread across queues so the first matmuls of a
                # block don't wait for one monolithic transfer
                t = xt_pool.tile([P, KD, TB], io_dt, name=f"xt{blk}", tag="xt")
                for k in range(KD):
                    nc.sync.dma_start(t[:, k], xt_r[:, k, ds(blk * TB, TB)])
                return t

            def load_w2_first(blk):
                # preload the first W2 chunk of every mq pass during phase A
                pre = []
                for mq in range(KD // MQ):
                    t = w2_pool.tile(
                        [P, FG2, MQ * P], io_dt, name=f"w2p{mq}", tag="w2"
                    )
                    nc.sync.dma_start(
                        t, w2_r[:, ds(0, FG2), ds(mq * MQ * P, MQ * P)]
                    )
                    pre.append(t)
                return pre

            xt_sb = load_xt(0)
            for blk in range(NB):
                h_sb = h_pool.tile([P, KF, TB], io_dt)
                w2_pre = None

                # ---- phase A: h^T[f, :] = gelu(W1^T x^T + b1) --------------
                for fg in range(KF // FG_):
                    w1_sb = w1_pool.tile([P, KD, FG_ * P], io_dt)
                    for k in range(KD):
                        nc.sync.dma_start(
                            w1_sb[:, k], w1_r[:, k, ds(fg * FG_ * P, FG_ * P)]
                        )
                    for fi in range(FG_):
                        f = fg * FG_ + fi
                        for nh in range(NH):
                            pa = psA.tile([P, MM_N], f32)
                            for k in range(KD):
                                nc.tensor.matmul(
                                    pa,
                                    mmcast(w1_sb[:, k, ds(fi * P, P)]),
                                    mmcast(xt_sb[:, k, ds(nh * MM_N, MM_N)]),
                                    start=(k == 0),
                                    stop=(k == KD - 1),
                                )
                            nc.scalar.activation(
                                h_sb[:, f, ds(nh * MM_N, MM_N)],
                                pa,
                                _ACT_FUNC,
                                bias=b1_sb[:, f : f + 1],
                            )
                    if fg == 1:
                        w2_pre = load_w2_first(blk)

                # next block's tokens: DMA starts as soon as phase A's last
                # read of the current xt slot retires, landing early in B
                xt_next = load_xt(blk + 1) if blk + 1 < NB else None

                # ---- phase B: y^T[m, :] = W2^T h^T + b2 --------------------
                for mq in range(KD // MQ):
                    pb = [
                        psB.tile([P, MM_N], f32, name=f"pb{i}", tag="pb")
                        for i in range(MQ * NH)
                    ]
                    for fg2 in range(KF // FG2):
                        if fg2 == 0:
                            w2_sb = w2_pre[mq]
                        else:
                            w2_sb = w2_pool.tile([P, FG2, MQ * P], io_dt)
                            nc.sync.dma_start(
                                w2_sb,
                                w2_r[:, ds(fg2 * FG2, FG2), ds(mq * MQ * P, MQ * P)],
                            )
                        for fi in range(FG2):
                            ffk = fg2 * FG2 + fi
                            for mm in range(MQ):
                                for nh in range(NH):
                                    nc.tensor.matmul(
                                        pb[mm * NH + nh],
                                        mmcast(w2_sb[:, fi, ds(mm * P, P)]),
                                        mmcast(h_sb[:, ffk, ds(nh * MM_N, MM_N)]),
                                        start=(ffk == 0),
                                        stop=(ffk == KF - 1),
                                    )
                    for mm in range(MQ):
                        m = mq * MQ + mm
                        for nh in range(NH):
                            y_sb = y_pool.tile([P, MM_N], f32)
                            nc.vector.tensor_scalar_add(
                                y_sb, pb[mm * NH + nh], b2_sb[:, m : m + 1]
                            )
                            nc.sync.dma_start(
                                yt_r[:, m, ds(blk * TB + nh * MM_N, MM_N)], y_sb
                            )
                xt_sb = xt_next
    if not nc.is_finalized():
        nc.finalize()
    return nc


def _get_nc(C: int, tag: str) -> bass.Bass:
    key = (C, tag)
    if key not in _NC_CACHE:
        _NC_CACHE[key] = _build(C, tag)
    return _NC_CACHE[key]


def kernel(x, gate_W, gate_b, W1, b1, W2, b2):
    global LAST_RESULTS
    x = np.asarray(x, np.float32)
    gate_W = np.asarray(gate_W, np.float32)
    gate_b = np.asarray(gate_b, np.float32)
    W1 = np.asarray(W1, np.float32)
    b1 = np.asarray(b1, np.float32)
    W2 = np.asarray(W2, np.float32)
    b2 = np.asarray(b2, np.float32)

    B, S, D = x.shape
    T = B * S
    xf = x.reshape(T, D)

    # ---- gate / routing (host) ------------------------------------------
    logits = xf @ gate_W + gate_b
    m = logits.max(-1, keepdims=True)
    ex = np.exp(logits - m)
    scores = ex / ex.sum(-1, keepdims=True)          # [T, NE] float32
    ar = np.arange(T)
    idx0 = scores.argmax(-1)
    tmp = scores.copy()
    tmp[ar, idx0] = -np.inf
    idx1 = tmp.argmax(-1)
    s0 = scores[ar, idx0]
    s1 = scores[ar, idx1]
    den = s0 + s1
    w0 = s0 / den
    w1w = s1 / den

    tok_idx = [None] * NE
    tok_w = [None] * NE
    for e in range(NE):
        m0 = idx0 == e
        m1 = idx1 == e
        ids = np.concatenate([ar[m0], ar[m1]])
        ws = np.concatenate([w0[m0], w1w[m1]]).astype(np.float32)
        tok_idx[e] = ids
        tok_w[e] = ws

    n_max = max(len(t) for t in tok_idx)
    C = TB * ((n_max + TB - 1) // TB)

    tag = MM_DTYPE
    io_np = np.float32
    if tag == "bf16":
        import ml_dtypes
        io_np = ml_dtypes.bfloat16

    # ---- build per-core inputs ------------------------------------------
    in_maps = []
    for e in range(NE):
        ids = tok_idx[e]
        XT = np.zeros((EMB, C), io_np)
        XT[:, : len(ids)] = xf[ids].T
        in_maps.append(
            {
                "xt": XT,
                "w1": np.ascontiguousarray(W1[e]).astype(io_np),
                "b1": np.ascontiguousarray(b1[e]),
                "w2": np.ascontiguousarray(W2[e]).astype(io_np),
                "b2": np.ascontiguousarray(b2[e]),
            }
        )

    nc = _get_nc(C, tag)
    res = run_bass_kernel_spmd(
        nc,
        in_maps,
        list(range(NE)),
        trace=PROFILE,
        trace_cores=list(range(NE)) if (PROFILE and PROFILE_ALL_CORES) else None,
    )
    if PROFILE:
        LAST_RESULTS = res

    # ---- combine (host) --------------------------------------------------
    out = np.zeros((T, EMB), np.float32)
    for e in range(NE):
        ids = tok_idx[e]
        n = len(ids)
        Y = res.results[e]["yt"][:, :n].T          # [n, EMB]
        out[ids] += tok_w[e][:, None] * Y

    # ---- aux loss (host) -------------------------------------------------
    counts = np.zeros(NE, np.float64)
    np.add.at(counts, idx0, 1.0)
    np.add.at(counts, idx1, 1.0)
    expert_fraction = counts / (T * TOPK)
    routing_weights = scores.mean(0, dtype=np.float64)
    aux_loss = np.float32(NE * np.sum(expert_fraction * routing_weights))

    return out.reshape(B, S, EMB), aux_loss


# revision 24
# speedup vs baseline: 1.1575x; 1.0783x over previous
"""MoE (8 experts, top-2) Trainium2 kernel.

Strategy (expert parallelism, per the sharding hint):
  - Host: compute the (tiny) gate: logits -> softmax -> top-2 -> normalized
    combine weights.  This is ~0.05% of total FLOPs.
  - Dispatch: for each expert e, gather the tokens routed to it (~2k of
    8192 * 2 slots), pad to a common capacity C, and send them (transposed,
    [EMB, C]) to core e together with expert e's weights.
  - Device (SPMD on 8 cores): dense FFN  y^T = W2^T @ gelu(W1^T @ x^T + b1) + b2
    computed block-by-block over tokens with all matmuls on the PE array.
  - Combine: host scales each expert's output rows by the combine weight and
    scatter-adds into the full output (each token receives exactly its top-2
    expert contributions).  aux_loss is reproduced on host from gate scores.

Only the FFN matmuls (99.9% of FLOPs) run on device; they are sized
[C,1024]x[1024,4096] + [C,4096]x[4096,1024] per core.
"""

import os
import numpy as np

import concourse.bass as bass
import concourse.mybir as mybir
from concourse import bacc
from concourse.bass import ds
from concourse.tile import TileContext
from concourse.bass_utils import run_bass_kernel_spmd

EMB = 1024
FF = 4096
NE = 8
TOPK = 2
P = 128

TB = 768        # tokens per block
MM_N = 384      # matmul moving free dim (TB == 2 * MM_N)
NH = TB // MM_N
KD = EMB // P   # 8 contraction tiles over EMB
KF = FF // P    # 32 contraction tiles over FF
FG = 2          # unused (phase A streams FG_ = 4 ff-tiles per chunk)
FG2 = 2         # ff-tiles of W2 per streamed chunk (phase B)
EH = 2          # emb halves per phase-B pass (N=512 moving chunks)
TT = TB // P    # token tiles per block (phase B stationary tiles)

# matmul precision: "f32" (exact, 4 cyc/row), "f32r" (fp32 storage,
# relaxed multiply, 1 cyc/row), "bf16" (bf16 storage+multiply, 1 cyc/row)
MM_DTYPE = os.environ.get("MOE_MM_DTYPE", "f32r")

_ACT_FUNC = mybir.ActivationFunctionType.Gelu  # swapped to Relu in sim tests

PROFILE = False          # set True (from test.py) to capture an NTFF trace
PROFILE_ALL_CORES = False
LAST_RESULTS = None      # BassKernelResults of the last run when PROFILE

_NC_CACHE = {}


def _build(C: int, tag: str) -> bass.Bass:
    f32 = mybir.dt.float32
    io_dt = {
        "bf16": mybir.dt.bfloat16,
        "f32r": mybir.dt.float32r,  # fp32 container, relaxed-precision multiply
        "f32": f32,
    }[tag]
    mmcast = lambda ap: ap

    NB = C // TB
    assert C % TB == 0
    bf16 = tag == "bf16"
    FG_ = 4                          # W1 stream chunk (ff-tiles)
    w1_bufs = 3 if bf16 else 2
    w2_bufs = 8 if bf16 else 6
    h_bufs = 2 if bf16 else 1

    nc = bacc.Bacc()
    xt_d = nc.declare_dram_parameter("xt", [EMB, C], io_dt, isOutput=False)
    w1_d = nc.declare_dram_parameter("w1", [EMB, FF], io_dt, isOutput=False)
    b1_d = nc.declare_dram_parameter("b1", [FF], f32, isOutput=False)
    w2_d = nc.declare_dram_parameter("w2", [FF, EMB], io_dt, isOutput=False)
    y_d = nc.declare_dram_parameter("y", [C, EMB], f32, isOutput=True)

    xt_r = xt_d[:].rearrange("(ko p) c -> p ko c", p=P)   # [128, 8, C]
    w1_r = w1_d[:].rearrange("(ko p) f -> p ko f", p=P)   # [128, 8, FF]
    w2_r = w2_d[:].rearrange("(ko p) e -> p ko e", p=P)   # [128, 32, EMB]
    b1_r = b1_d[:].rearrange("(t p) -> p t", p=P)         # [128, 32]
    y_r = y_d[:].rearrange("(to p) e -> p to e", p=P)     # [128, C/128, EMB]

    with TileContext(nc) as tc:
        with (
            tc.tile_pool(name="const", bufs=1) as cpool,
            tc.tile_pool(name="xt", bufs=1) as xt_pool,
            tc.tile_pool(name="w1", bufs=w1_bufs) as w1_pool,
            tc.tile_pool(name="w2", bufs=w2_bufs) as w2_pool,
            tc.tile_pool(name="h", bufs=h_bufs) as h_pool,
            tc.tile_pool(name="y", bufs=4) as y_pool,
            tc.tile_pool(name="psA", bufs=2, space="PSUM") as psA,
            tc.tile_pool(name="psB", bufs=6, space="PSUM") as psB,
        ):
            b1_sb = cpool.tile([P, KF], f32)
            nc.sync.dma_start(b1_sb, b1_r)

            def load_xt(blk):
                # per-k DMAs spread across queues so the first matmuls of a
                # block don't wait for one monolithic transfer
                t = xt_pool.tile([P, KD, TB], io_dt, name=f"xt{blk}", tag="xt")
                for k in range(KD):
                    nc.sync.dma_start(t[:, k], xt_r[:, k, ds(blk * TB, TB)])
                return t

            def load_w2_chunk(fg2, half, name):
                # [P, FG2 ffk-tiles, 512 emb] moving-operand chunk, split per
                # ffk row so the transfers spread across DMA queues
                t = w2_pool.tile([P, FG2, EMB // EH], io_dt, name=name, tag="w2")
                for fi in range(FG2):
                    nc.sync.dma_start(
                        t[:, fi],
                        w2_r[:, fg2 * FG2 + fi, ds(half * (EMB // EH), EMB // EH)],
                    )
                return t

            def load_w2_first(blk):
                # preload the first W2 chunk of each emb-half pass during A
                return [
                    load_w2_chunk(0, half, f"w2p{half}") for half in range(EH)
                ]

            xt_sb = load_xt(0)
            for blk in range(NB):
                h_sb = h_pool.tile([P, KF, TB], io_dt)
                w2_pre = None

                # ---- phase A: h^T[f, :] = gelu(W1^T x^T + b1) --------------
                for fg in range(KF // FG_):
                    w1_sb = w1_pool.tile([P, KD, FG_ * P], io_dt)
                    for k in range(KD):
                        nc.sync.dma_start(
                            w1_sb[:, k], w1_r[:, k, ds(fg * FG_ * P, FG_ * P)]
                        )
                    for fi in range(FG_):
                        f = fg * FG_ + fi
                        for nh in range(NH):
                            pa = psA.tile([P, MM_N], f32)
                            for k in range(KD):
                                nc.tensor.matmul(
                                    pa,
                                    mmcast(w1_sb[:, k, ds(fi * P, P)]),
                                    mmcast(xt_sb[:, k, ds(nh * MM_N, MM_N)]),
                                    start=(k == 0),
                                    stop=(k == KD - 1),
                                )
                            nc.scalar.activation(
                                h_sb[:, f, ds(nh * MM_N, MM_N)],
                                pa,
                                _ACT_FUNC,
                                bias=b1_sb[:, f : f + 1],
                            )
                    if fg == 1:
                        w2_pre = load_w2_first(blk)

                # next block's tokens: DMA starts as soon as phase A's last
                # read of the current xt slot retires, landing early in B
                xt_next = load_xt(blk + 1) if blk + 1 < NB else None

                # ---- phase B: y[t, :] = h W2  (swapped operands) -----------
                # stationary = h^T tile [ff128, tok128], moving = W2 [ff128,
                # emb512] in natural layout -> matmul-bound at N=512 instead
                # of LDWEIGHTS-bound; b2 is added on the host
                for half in range(EH):
                    pb = [
                        psB.tile([P, EMB // EH], f32, name=f"pb{t}", tag="pb")
                        for t in range(TT)
                    ]
                    for fg2 in range(KF // FG2):
                        if fg2 == 0:
                            w2_sb = w2_pre[half]
                        else:
                            w2_sb = load_w2_chunk(fg2, half, "w2s")
                        for fi in range(FG2):
                            ffk = fg2 * FG2 + fi
                            for t in range(TT):
                                nc.tensor.matmul(
                                    pb[t],
                                    mmcast(h_sb[:, ffk, ds(t * P, P)]),
                                    mmcast(w2_sb[:, fi]),
                                    start=(ffk == 0),
                                    stop=(ffk == KF - 1),
                                )
                    for t in range(TT):
                        y_sb = y_pool.tile([P, EMB // EH], f32)
                        nc.any.tensor_copy(y_sb, pb[t])
                        nc.sync.dma_start(
                            y_r[:, blk * TT + t, ds(half * (EMB // EH), EMB // EH)],
                            y_sb,
                        )
                xt_sb = xt_next
    if not nc.is_finalized():
        nc.finalize()
    return nc


def _get_nc(C: int, tag: str) -> bass.Bass:
    key = (C, tag)
    if key not in _NC_CACHE:
        _NC_CACHE[key] = _build(C, tag)
    return _NC_CACHE[key]


def kernel(x, gate_W, gate_b, W1, b1, W2, b2):
    global LAST_RESULTS
    x = np.asarray(x, np.float32)
    gate_W = np.asarray(gate_W, np.float32)
    gate_b = np.asarray(gate_b, np.float32)
    W1 = np.asarray(W1, np.float32)
    b1 = np.asarray(b1, np.float32)
    W2 = np.asarray(W2, np.float32)
    b2 = np.asarray(b2, np.float32)

    B, S, D = x.shape
    T = B * S
    xf = x.reshape(T, D)

    # ---- gate / routing (host) ------------------------------------------
    logits = xf @ gate_W + gate_b
    m = logits.max(-1, keepdims=True)
    ex = np.exp(logits - m)
    scores = ex / ex.sum(-1, keepdims=True)          # [T, NE] float32
    ar = np.arange(T)
    idx0 = scores.argmax(-1)
    tmp = scores.copy()
    tmp[ar, idx0] = -np.inf
    idx1 = tmp.argmax(-1)
    s0 = scores[ar, idx0]
    s1 = scores[ar, idx1]
    den = s0 + s1
    w0 = s0 / den
    w1w = s1 / den

    tok_idx = [None] * NE
    tok_w = [None] * NE
    for e in range(NE):
        m0 = idx0 == e
        m1 = idx1 == e
        ids = np.concatenate([ar[m0], ar[m1]])
        ws = np.concatenate([w0[m0], w1w[m1]]).astype(np.float32)
        tok_idx[e] = ids
        tok_w[e] = ws

    n_max = max(len(t) for t in tok_idx)
    C = TB * ((n_max + TB - 1) // TB)

    tag = MM_DTYPE
    io_np = np.float32
    if tag == "bf16":
        import ml_dtypes
        io_np = ml_dtypes.bfloat16

    # ---- build per-core inputs ------------------------------------------
    in_maps = []
    for e in range(NE):
        ids = tok_idx[e]
        XT = np.zeros((EMB, C), io_np)
        XT[:, : len(ids)] = xf[ids].T
        in_maps.append(
            {
                "xt": XT,
                "w1": np.ascontiguousarray(W1[e]).astype(io_np),
                "b1": np.ascontiguousarray(b1[e]),
                "w2": np.ascontiguousarray(W2[e]).astype(io_np),
            }
        )

    nc = _get_nc(C, tag)
    res = run_bass_kernel_spmd(
        nc,
        in_maps,
        list(range(NE)),
        trace=PROFILE,
        trace_cores=list(range(NE)) if (PROFILE and PROFILE_ALL_CORES) else None,
    )
    if PROFILE:
        LAST_RESULTS = res

    # ---- combine (host) --------------------------------------------------
    out = np.zeros((T, EMB), np.float32)
    for e in range(NE):
        ids = tok_idx[e]
        n = len(ids)
        Y = res.results[e]["y"][:n] + b2[e]        # [n, EMB]; b2 added here
        out[ids] += tok_w[e][:, None] * Y

    # ---- aux loss (host) -------------------------------------------------
    counts = np.zeros(NE, np.float64)
    np.add.at(counts, idx0, 1.0)
    np.add.at(counts, idx1, 1.0)
    expert_fraction = counts / (T * TOPK)
    routing_weights = scores.mean(0, dtype=np.float64)
    aux_loss = np.float32(NE * np.sum(expert_fraction * routing_weights))

    return out.reshape(B, S, EMB), aux_loss
